# revision 7
# baseline (speedup 1.0000x reference)
"""Trainium2 Bass kernel for nn_CustomActivation (fp16 I/O):

    out[b, d] = sum_k alpha[k, d % 64] * relu(x[b, d] + gamma[k, d % 64])

x: [8192, 4096] f32, alpha/gamma: [3, 64] f32.

Strategy
--------
Shard x along FEATURE columns across 8 cores (512 each), transposed host-side
so on-chip layout is [partition = d, free = b] and the [3, 64] params become
per-partition scalars / diagonal matrices (the d-range of every 128-partition
block is a multiple of 64, so one [128] vector serves all blocks and cores).

All device I/O is FP16: halves HBM traffic vs f32.  The f32 baseline sat at
the ~358 GB/s/core HBM roofline (measured ~123 us/core with the hw-loop
method); fp16 lowers the DMA floor to ~48.5 us/core measured.

Engine split, per [128, 8192] row-block, chunked 2048 wide (all stages
under the DMA budget; ACT relu would be 1 elem/cyc/lane dtype-independent
= a ~90 us floor, so relus go to DVE and the combine to the idle TensorE):
  DMA : in/out chunks on the SP HWDGE ring (4 KB runs per partition-row)
  DVE : t_k = (x add g_k) max 0     tensor_scalar, fp16 4x mode, 1024-wide
        sub-passes so the first matmul group starts early
  PE  : psum[:, j] += diag(a_k) @ t_k[:, j]   fp16 matmuls, 512-wide (PSUM
        bank cap: s3d3_mm_num_elements), j-OUTER order: each bank region
        completes after its 3 k-matmuls so ACT copies chase the PE
  ACT : out_sb(fp16) = copy(psum[:, j])  per-bank fp32->fp16 convert

Measured (hw-loop slope, n_rep 33 vs 1025, paired-med, same window):
j-outer+fine 65.0-65.5 vs k-outer 70.8-71.6 vs f32 baseline 104-124 us.
Device drifts ~20% between windows; fast-window best ~59-60 us.
Ablation floors: dmaonly 48.5, compute-only 50, independent dma+compute
52.5 us.
"""

import numpy as np

import concourse.bacc as bacc
import concourse.mybir as mybir
from concourse.tile import TileContext

N_CORES = 8
B, D, L = 8192, 4096, 64
DS = D // N_CORES  # 512 feature columns per core
P = 128

F16 = mybir.dt.float16
F32 = mybir.dt.float32

CONFIG = dict(mm_fd=512, q_fd=2048, f_w=8192)


def build_program(
    ds: int = DS,
    b: int = B,
    n_rep: int = 1,
    mm_fd: int | None = None,
    q_fd: int | None = None,
    f_w: int | None = None,
):
    """SPMD Bass program one core runs on its [ds, b] fp16 shard.

    n_rep > 1 wraps the pass in a hardware loop (tc.For_i) for benchmarking.
    """
    from contextlib import nullcontext

    mm_fd = mm_fd or CONFIG["mm_fd"]
    q_fd = q_fd or CONFIG["q_fd"]
    f_w = f_w or CONFIG["f_w"]
    nc = bacc.Bacc("TRN2", target_bir_lowering=False, debug=False)

    xT = nc.dram_tensor("xT", [ds, b], F16, kind="ExternalInput").ap()
    pv = nc.dram_tensor("pv", [P, 4], F32, kind="ExternalInput").ap()
    wts = nc.dram_tensor("wts", [P, 3 * P], F16, kind="ExternalInput").ap()
    oT = nc.dram_tensor("oT", [ds, b], F16, kind="ExternalOutput").ap()

    n_blk = ds // P
    n_f = b // f_w
    n_q = f_w // q_fd
    n_j = q_fd // mm_fd
    A = mybir.AluOpType

    with TileContext(nc) as tc:
        with (
            tc.tile_pool(name="params", bufs=1) as ppool,
            tc.tile_pool(name="xin", bufs=8) as xpool,
            tc.tile_pool(name="t0", bufs=6) as t0pool,
            tc.tile_pool(name="t1", bufs=6) as t1pool,
            tc.tile_pool(name="t2", bufs=6) as t2pool,
            tc.tile_pool(name="out", bufs=6) as opool,
            tc.tile_pool(name="ps", bufs=2, space="PSUM") as pspool,
        ):
            p_s = ppool.tile([P, 4], F32)
            nc.sync.dma_start(out=p_s, in_=pv)
            w_s = ppool.tile([P, 3 * P], F16)
            nc.sync.dma_start(out=w_s, in_=wts)
            g = [p_s[:, k : k + 1] for k in range(3)]
            w = [w_s[:, k * P : (k + 1) * P] for k in range(3)]

            rep_ctx = tc.For_i(0, n_rep) if n_rep > 1 else nullcontext()
            with rep_ctx:
                # chunk-width tiles with deep pools: short tile lifetimes
                # give every stage several chunks of in-flight slack
                for blk in range(n_blk):
                    sl0 = slice(blk * P, (blk + 1) * P)
                    for q in range(b // q_fd):
                        sq = slice(q * q_fd, (q + 1) * q_fd)
                        xc = xpool.tile([P, q_fd], F16)
                        nc.sync.dma_start(out=xc, in_=xT[sl0, sq])
                        t_0 = t0pool.tile([P, q_fd], F16)
                        t_1 = t1pool.tile([P, q_fd], F16)
                        t_2 = t2pool.tile([P, q_fd], F16)
                        tsq = [t_0, t_1, t_2]
                        # relus at 1024 granularity: the first matmul
                        # group starts ~1 us earlier than full-chunk relu
                        for r0 in range(0, q_fd, 1024):
                            rs = slice(r0, r0 + 1024)
                            for k in range(3):
                                nc.vector.tensor_scalar(
                                    tsq[k][:, rs], xc[:, rs], g[k],
                                    0.0, A.add, A.max,
                                )
                        ps = pspool.tile([P, q_fd], F32)
                        oc = opool.tile([P, q_fd], F16)
                        # j-outer: each 512-wide PSUM bank region is
                        # complete after its 3 k-matmuls, so the ACT
                        # copies chase the PE at bank granularity
                        # instead of waiting for the whole 12-MM group
                        for j in range(n_j):
                            fj = slice(j * mm_fd, (j + 1) * mm_fd)
                            for k in range(3):
                                nc.tensor.matmul(
                                    ps[:, fj],
                                    w[k],
                                    tsq[k][:, fj],
                                    start=(k == 0),
                                    stop=(k == 2),
                                )
                            nc.scalar.copy(out=oc[:, fj], in_=ps[:, fj])
                        nc.sync.dma_start(out=oT[sl0, sq], in_=oc)
    nc.compile()
    return nc


def _host_params(alpha: np.ndarray, gamma: np.ndarray):
    """pv [128,4] fp32 (g0,g1,g2,0) and wts [128, 384] fp16 (3 diag mats)."""
    a = np.tile(np.asarray(alpha, np.float32), (1, P // L))  # [3, 128]
    g = np.tile(np.asarray(gamma, np.float32), (1, P // L))
    pv = np.zeros((P, 4), np.float32)
    pv[:, :3] = g.T
    wts = np.zeros((P, 3 * P), np.float16)
    for k in range(3):
        wts[:, k * P : (k + 1) * P] = np.diag(a[k]).astype(np.float16)
    return pv, wts


def make_in_maps(inputs: dict) -> list:
    pv, wts = _host_params(inputs["alpha"], inputs["gamma"])
    xT = np.ascontiguousarray(
        np.asarray(inputs["x"], dtype=np.float32).T.astype(np.float16)
    )
    return [
        {"xT": xT[c * DS : (c + 1) * DS], "pv": pv, "wts": wts}
        for c in range(N_CORES)
    ]


_program_cache: dict = {}


def kernel(x: np.ndarray, alpha: np.ndarray, gamma: np.ndarray) -> np.ndarray:
    from concourse.bass_utils import run_bass_kernel_spmd

    pv, wts = _host_params(alpha, gamma)
    xT = np.ascontiguousarray(
        np.asarray(x, dtype=np.float32).T.astype(np.float16)
    )  # [D, B] fp16
    if "nc" not in _program_cache:
        _program_cache["nc"] = build_program()
    nc = _program_cache["nc"]
    in_maps = [
        {"xT": xT[c * DS : (c + 1) * DS], "pv": pv, "wts": wts}
        for c in range(N_CORES)
    ]
    res = run_bass_kernel_spmd(nc, in_maps, core_ids=list(range(N_CORES)))
    oT = np.concatenate([r["oT"] for r in res.results], axis=0)  # [D, B] fp16
    return np.ascontiguousarray(oT.T.astype(np.float32))
